# revision 29
# baseline (speedup 1.0000x reference)
"""Self-contained Trainium2 Bass kernel for nn_Attention_59253368816224.

GQA attention block: per-head RMSNorm on Q/K, RoPE, causal softmax
attention, o_proj.  B=2, S=2048, H=2048, 16 Q heads / 4 KV heads,
head_dim=128.

Sharding: 8 cores = 2 batches x 4 KV groups.  Core c -> (b=c//4, g=c%4)
owns 4 Q heads + 1 KV head.  o_proj is row-parallel: each core emits a
partial [S, H] output contracted over its 512 hidden dims; the host sums
the 4 partials per batch in fp32.

Device pipeline (matmuls bf16, fp32 PSUM accumulation):
  Phase A runs in 4 quarter-passes over the sequence (512 cols each),
  each split into two 3-chunk half-passes with a k-inner loop so the
  startup is DMA-paced.  Per chunk: PSUM copy, sum-of-squares via
  ones-matmul, RMS scale rsqrt(mean+eps) = exp(-0.5 ln(.)) on ACT
  (single natural_log_exp table set for the whole kernel), RoPE with
  the norm scale applied to the source and the sin table pre-rotated so
  the rotation matmul feeds a single fused add.  v quarters go through
  a DRAM round-trip transpose into natural layout.

  Attention i-chunk ic only needs sequence quarter ic of qfin plus
  quarters 0..ic of kfin/v3, so attention chunk ic-1 is emitted
  interleaved with pass ic (staircase overlap): the PE runs projection
  matmuls while ACT chews exp tiles and vice versa.  Scores land
  transposed [j, i]; exp on ACT; the causal mask multiply touches only
  the 128 triangular boundary columns; the softmax denominator is
  accumulated tile-by-tile on DVE/GpSimd into an fp32 S tile and
  reduced across partitions with one fp32 ones-matmul; normalization
  uses DVE reciprocal (no ACT table swap).  o_proj partials stream out
  per 128-row tile as soon as all four heads of an i-chunk are done.

  x is staged in sequence halves: the first-half tiles are freed after
  pass 1 and the second-half pool reuses their SBUF.
"""

import os
import sys
import numpy as np
import ml_dtypes

BF16 = ml_dtypes.bfloat16

B = 2
S = 2048
H = 2048
NQH = 16          # total q heads
NKV = 4           # total kv heads
HD = 128          # head dim
GQ = 4            # q heads per core (per kv group)
KT = H // 128     # 16 k-tiles over hidden
ST = S // 128     # 16 s-tiles
RMS_EPS = 1.1920928955078125e-07
INV_SQRT_HD = 1.0 / float(np.sqrt(HD))

_PROGRAM = None


def _patch_act_tables():
    """Route both Exp and Ln to the natural_log_exp_and_others table set.

    The act-table insertion pass picks the first set containing each
    function; by default Exp resolves to exp_and_others and Ln to
    natural_log, so alternating Exp/Ln thrashes ACT_TABLE_LOAD (~1.5us
    each).  Removing Exp/Ln from the earlier sets makes both resolve to
    the combined set -> one table load for the whole kernel.
    """
    import concourse.bacc as bacc
    import concourse.hw_specs as hw_specs
    from concourse import mybir

    if getattr(bacc, "_act_tables_patched", False):
        return
    orig = hw_specs.get_activation_tables

    def patched(arch):
        t = dict(orig(arch))
        out = {}
        for name, fns in t.items():
            fns = set(fns)
            if name == "exp_and_others":
                fns.discard(mybir.ActivationFunctionType.Exp)
            if name == "natural_log":
                fns.discard(mybir.ActivationFunctionType.Ln)
            out[name] = fns
        return out

    bacc.get_activation_tables = patched
    bacc._act_tables_patched = True


def _build_program():
    import concourse.bacc as bacc
    import concourse.tile as tile
    from concourse import mybir
    from contextlib import ExitStack

    _patch_act_tables()

    bf = mybir.dt.bfloat16
    f32 = mybir.dt.float32

    nc = bacc.Bacc("TRN2", target_bir_lowering=False, debug=False, num_devices=8)

    # ---- DRAM I/O (per-core values supplied via in_maps) ----
    xt_d = nc.dram_tensor("xt", (H, S), bf, kind="ExternalInput")
    wq_d = nc.dram_tensor("wq", (128, KT * 512), bf, kind="ExternalInput")
    wk_d = nc.dram_tensor("wk", (128, KT * 128), bf, kind="ExternalInput")
    wv_d = nc.dram_tensor("wv", (128, KT * 128), bf, kind="ExternalInput")
    wo_d = nc.dram_tensor("wo", (128, GQ * H), bf, kind="ExternalInput")
    cosq_d = nc.dram_tensor("cosq", (HD, S), bf, kind="ExternalInput")
    sinq_d = nc.dram_tensor("sinq", (HD, S), bf, kind="ExternalInput")
    cosk_d = nc.dram_tensor("cosk", (HD, S), bf, kind="ExternalInput")
    sink_d = nc.dram_tensor("sink", (HD, S), bf, kind="ExternalInput")
    cb_d = nc.dram_tensor("cb", (128, 3 * 128), bf, kind="ExternalInput")
    onesf_d = nc.dram_tensor("onesf", (128, 128), mybir.dt.float32,
                             kind="ExternalInput")
    out_d = nc.dram_tensor("out", (S, H), bf, kind="ExternalOutput")
    # internal scratch for the per-quarter v transpose (salted name so
    # experiments can bust the NEFF cache)
    vt_scratch = nc.dram_tensor(
        "vt_scratch" + os.environ.get("KERNEL_SALT", ""), (HD, S), bf)

    Exp = mybir.ActivationFunctionType.Exp
    Ln = mybir.ActivationFunctionType.Ln

    GROUP_A = [4, 5, 0]     # k, v, q-head 0
    GROUP_B = [1, 2, 3]     # q heads 1..3

    with tile.TileContext(nc) as tc:
        with ExitStack() as ctx:
            consts = ctx.enter_context(tc.tile_pool(name="consts", bufs=1))
            persist = ctx.enter_context(tc.tile_pool(name="persist", bufs=1))

            # ---- persistent intermediates (live into attention) ----
            qfin = persist.tile([128, GQ, S], bf)      # roped+normed qT
            kfin = persist.tile([128, S], bf)          # roped+normed kT
            v3 = persist.tile([128, ST, HD], bf)       # v natural [jt][j][d]

            # ---- constant tiles ----
            cosq = consts.tile([128, S], bf)
            sinq = consts.tile([128, S], bf)
            cosk = consts.tile([128, S], bf)
            sink = consts.tile([128, S], bf)
            cb = consts.tile([128, 3, 128], bf)        # rmat | tri | ones
            onesf = consts.tile([128, 128], mybir.dt.float32)
            eps128 = consts.tile([128, 1], mybir.dt.float32)
            nc.vector.memset(eps128[:], RMS_EPS)
            rmat = cb[:, 0, :]
            tri = cb[:, 1, :]
            onesb = cb[:, 2, :]

            with ExitStack() as actx:
                # ---------- phase A pools ----------
                proj_w = actx.enter_context(tc.tile_pool(name="proj_w", bufs=1))
                rawp = actx.enter_context(tc.tile_pool(name="rawp", bufs=6))
                sqp = actx.enter_context(tc.tile_pool(name="sqp", bufs=3))
                scbp = actx.enter_context(tc.tile_pool(name="scbp", bufs=2))
                ropep = actx.enter_context(tc.tile_pool(name="ropep", bufs=2))
                # PSUM: qkv 3 + sr 1 + sc 2 + acc 2 = 8 banks
                qkv_ps = actx.enter_context(
                    tc.tile_pool(name="qkv_ps", bufs=3, space="PSUM"))
                sr_ps = actx.enter_context(
                    tc.tile_pool(name="sr_ps", bufs=1, space="PSUM"))

                # ---------- attention pools ----------
                wop = actx.enter_context(tc.tile_pool(name="wop", bufs=1))
                attp = actx.enter_context(tc.tile_pool(name="attnT", bufs=32))
                attS = actx.enter_context(tc.tile_pool(name="attS", bufs=4))
                rnp = actx.enter_context(tc.tile_pool(name="rnorm", bufs=1))
                ostage = actx.enter_context(tc.tile_pool(name="ostage", bufs=2))
                otp = actx.enter_context(tc.tile_pool(name="otsb", bufs=1))
                sc_psp = actx.enter_context(
                    tc.tile_pool(name="sc_ps", bufs=2, space="PSUM"))
                acc_psp = actx.enter_context(
                    tc.tile_pool(name="acc_ps", bufs=2, space="PSUM"))

                otsb = otp.tile([128, GQ, S], bf)      # oT per head

                # ---------- weight tiles + DMAs (priority order) ----------
                wq_all = proj_w.tile([128, GQ, KT, 128], bf)  # head-major
                wk_all = proj_w.tile([128, KT, 128], bf)
                wv_all = proj_w.tile([128, KT, 128], bf)
                wo_sb = wop.tile([128, GQ, H], bf)

                def lhsT_of(c, k):
                    if c < 4:
                        return wq_all[:, c, k, :]
                    elif c == 4:
                        return wk_all[:, k, :]
                    else:
                        return wv_all[:, k, :]

                # ---------- phase A helpers ----------
                # rms+rope post-work is deferred and emitted inside later
                # k-loops / produce calls so its matmuls never stall the PE
                deferred = []

                def post_piece1(q, c, raw, sq):
                    """ss matmul + scale + rope elementwise chain.

                    The rotate-half is a 64-partition roll: done with two
                    SBUF->SBUF DMAs (sign pre-folded into the sin table),
                    so RoPE costs the PE nothing.
                    """
                    s0 = q * 512
                    sst = sr_ps.tile([128, 512], f32, tag="sr",
                                     name=f"ss_{q}_{c}")
                    nc.tensor.matmul(sst[:], onesb, sq[:],
                                     start=True, stop=True)
                    scb = scbp.tile([128, 512], f32, tag="scb")
                    nc.scalar.activation(scb[:], sst[:], Ln,
                                         bias=eps128[:], scale=1.0 / HD)
                    nc.scalar.activation(scb[:], scb[:], Exp, scale=-0.5)
                    cosx = cosq if c < 4 else cosk
                    sinx = sinq if c < 4 else sink
                    qs = ropep.tile([128, 512], bf, tag="qs")
                    aa = ropep.tile([128, 512], bf, tag="aa")
                    bb = ropep.tile([128, 512], bf, tag="bb")
                    br = ropep.tile([128, 512], bf, tag="br")
                    nc.vector.tensor_mul(qs[:], raw[:], scb[:])
                    nc.vector.tensor_mul(aa[:], qs[:], cosx[:, s0:s0 + 512])
                    nc.vector.tensor_mul(bb[:], qs[:], sinx[:, s0:s0 + 512])
                    nc.gpsimd.dma_start(out=br[0:64, :], in_=bb[64:128, :])
                    nc.gpsimd.dma_start(out=br[64:128, :], in_=bb[0:64, :])
                    return aa, br

                def post_piece2(q, c, aa, br):
                    """final add (emitted a few slots after piece1 so the
                    chain and the roll DMAs are already done)."""
                    s0 = q * 512
                    fin = qfin[:, c, s0:s0 + 512] if c < 4 \
                        else kfin[:, s0:s0 + 512]
                    nc.vector.tensor_add(fin, aa[:], br[:])

                def pop_deferred():
                    if deferred:
                        deferred.pop(0)()

                def flush_deferred():
                    while deferred:
                        pop_deferred()

                def defer_post(q, c, raw, sq):
                    def p1():
                        aa, bb = post_piece1(q, c, raw, sq)
                        deferred.append(lambda: post_piece2(q, c, aa, bb))
                    deferred.append(p1)

                def half_pass(q, group, xtiles):
                    s0 = q * 512
                    xo = (q % 2) * 512
                    ps = {}
                    for c in group:
                        ps[c] = qkv_ps.tile([128, 512], f32, tag="ps",
                                            name=f"ps_{q}_{c}")
                    for k in range(KT):
                        for c in group:
                            nc.tensor.matmul(
                                ps[c][:],
                                lhsT_of(c, k),
                                xtiles[k][:, xo:xo + 512],
                                start=(k == 0),
                                stop=(k == KT - 1),
                            )
                        if k in (2, 5, 8, 11, 14):
                            pop_deferred()
                    for ci, c in enumerate(group):
                        raw = rawp.tile([128, 512], bf, tag="raw",
                                        name=f"raw_{q}_{c}")
                        if c == 5:
                            # v quarter: copy on DVE, then DMA round-trip
                            # transpose into v3 (triggers off the sync queue
                            # so input loads are never blocked)
                            nc.vector.tensor_copy(raw[:], ps[c][:])
                            nc.gpsimd.dma_start(
                                out=vt_scratch[:, s0:s0 + 512], in_=raw[:])
                            nc.scalar.dma_start_transpose(
                                out=v3[:, 4 * q:4 * q + 4, :],
                                in_=vt_scratch[:, s0:s0 + 512])
                            continue
                        if ci % 2 == 0:
                            nc.scalar.copy(raw[:], ps[c][:])
                        else:
                            nc.vector.tensor_copy(raw[:], ps[c][:])
                        sq = sqp.tile([128, 512], bf, tag="sq")
                        nc.gpsimd.tensor_mul(sq[:], raw[:], raw[:])
                        defer_post(q, c, raw, sq)

                # ---------- attention helpers ----------
                # Emission is piece-wise: produce pieces (scores MM -> exp ->
                # mask -> S-chain add) are interleaved with the previous
                # head's consume (attn@v) matmuls and pending o_proj blocks
                # so the PE queue always has fill work while ACT chews exps.

                def produce_pieces(ic, h):
                    """Yield per-jt closures; state dict carries ats/S."""
                    i0 = ic * 512
                    offd = list(range(4 * ic))
                    jt_order = [4 * ic] + offd + \
                        [4 * ic + 1, 4 * ic + 2, 4 * ic + 3]
                    gp_set = set(offd[1::2])
                    state = {"ats": {}, "Sv": None, "Sg": None}

                    def piece(jt):
                        t = max(jt - 4 * ic, 0)
                        w = 512 - t * 128
                        at = attp.tile([128, 512], bf, tag="at",
                                       name=f"at_{ic}_{h}_{jt}")
                        sc = sc_psp.tile([128, 512], f32, tag="sc",
                                         name=f"sc_{ic}_{h}_{jt}")
                        nc.tensor.matmul(
                            sc[:, :w],
                            kfin[:, jt * 128:(jt + 1) * 128],
                            qfin[:, h, i0 + t * 128:i0 + 512],
                            start=True, stop=True)
                        nc.scalar.activation(at[:, t * 128:], sc[:, :w], Exp,
                                             scale=INV_SQRT_HD)
                        if jt >= 4 * ic:
                            # triangular boundary only spans 128 cols
                            nc.gpsimd.tensor_mul(
                                at[:, t * 128:(t + 1) * 128],
                                at[:, t * 128:(t + 1) * 128],
                                tri)
                        if jt in gp_set:
                            if state["Sg"] is None:
                                Sg = attS.tile([128, 512], f32, tag="S",
                                               name=f"Sg_{ic}_{h}")
                                nc.vector.tensor_copy(Sg[:], at[:])
                                state["Sg"] = Sg
                            else:
                                nc.gpsimd.tensor_add(state["Sg"][:],
                                                     state["Sg"][:], at[:])
                        else:
                            if state["Sv"] is None:
                                Sv = attS.tile([128, 512], f32, tag="S",
                                               name=f"Sv_{ic}_{h}")
                                nc.vector.tensor_copy(Sv[:], at[:])
                                state["Sv"] = Sv
                            else:
                                nc.vector.tensor_add(
                                    state["Sv"][:, t * 128:],
                                    state["Sv"][:, t * 128:],
                                    at[:, t * 128:])
                        state["ats"][jt] = at

                    return [lambda jt=jt: piece(jt) for jt in jt_order], state

                def consume_pieces(ic, h, state):
                    """Closures: njt attn@v matmuls then rowsum+normalize."""
                    i0 = ic * 512
                    njt = 4 * ic + 4
                    box = {}

                    def av(jt):
                        if jt == 0:
                            box["ot"] = acc_psp.tile(
                                [128, 512], f32, tag="acc",
                                name=f"ot_{ic}_{h}")
                        t = max(jt - 4 * ic, 0) * 128
                        nc.tensor.matmul(
                            box["ot"][:, t:],
                            v3[:, jt, :],
                            state["ats"][jt][:, t:],
                            start=(jt == 0),
                            stop=(jt == njt - 1),
                            skip_group_check=True,
                        )

                    def norm():
                        Sv, Sg = state["Sv"], state["Sg"]
                        rs = acc_psp.tile([128, 512], f32, tag="acc",
                                          name=f"rs_{ic}_{h}")
                        nc.tensor.matmul(rs[:], onesf[:], Sv[:],
                                         start=True, stop=(Sg is None))
                        if Sg is not None:
                            nc.tensor.matmul(rs[:], onesf[:], Sg[:],
                                             start=False, stop=True)
                        # 1/rs = exp(-ln(rs)); same ACT table set
                        lnr = rnp.tile([128, 512], f32, tag="lnr")
                        rr = rnp.tile([128, 512], f32, tag="rr")
                        nc.scalar.activation(lnr[:], rs[:], Ln)
                        nc.scalar.activation(rr[:], lnr[:], Exp, scale=-1.0)
                        nc.vector.tensor_mul(otsb[:, h, i0:i0 + 512],
                                             box["ot"][:], rr[:])

                    return [lambda jt=jt: av(jt) for jt in range(njt)] + [norm]

                def oproj_pieces(m):
                    """Closures: 4 nn-blocks (4 MMs + copy + piece DMA)."""
                    box = {}

                    def block(nn):
                        if nn == 0:
                            box["ob"] = ostage.tile([128, H], bf, tag="ob",
                                                    name=f"ob{m}")
                        op = acc_psp.tile([128, 512], f32, tag="acc",
                                          name=f"op{m}_{nn}")
                        for h in range(GQ):
                            nc.tensor.matmul(
                                op[:],
                                otsb[:, h, m * 128:(m + 1) * 128],
                                wo_sb[:, h, nn * 512:(nn + 1) * 512],
                                start=(h == 0),
                                stop=(h == GQ - 1),
                            )
                        sl = slice(nn * 512, (nn + 1) * 512)
                        # GpSimd cannot read PSUM; split DVE/ACT, and DMA
                        # each piece off the queue that produced it
                        if nn % 2 == 0:
                            nc.vector.tensor_copy(box["ob"][:, sl], op[:])
                            nc.gpsimd.dma_start(
                                out=out_d[m * 128:(m + 1) * 128, sl],
                                in_=box["ob"][:, sl])
                        else:
                            nc.scalar.copy(box["ob"][:, sl], op[:])
                            nc.scalar.dma_start(
                                out=out_d[m * 128:(m + 1) * 128, sl],
                                in_=box["ob"][:, sl])

                    return [lambda nn=nn: block(nn) for nn in range(4)]

                # fill queue: consume/oproj closures appended as they become
                # legal, popped between produce pieces to keep the PE fed
                fill = []
                pending = []   # oproj m-tiles whose 4 heads are consumed

                def refill():
                    while len(fill) < 8 and pending:
                        fill.extend(oproj_pieces(pending.pop(0)))

                def emit_interleaved(ppieces):
                    refill()
                    for p in ppieces:
                        p()
                        for _ in range(2):
                            if fill:
                                fill.pop(0)()
                        refill()

                def att_part(ic, step):
                    if ic < 0:
                        return
                    if step == 0:
                        flush_deferred()   # quarter ic rope must be emitted
                        pp, st = produce_pieces(ic, 0)
                        att_part.live[0] = st
                        emit_interleaved(pp)
                    elif step in (1, 2, 3):
                        fill.extend(
                            consume_pieces(ic, step - 1,
                                           att_part.live[step - 1]))
                        pp, st = produce_pieces(ic, step)
                        att_part.live[step] = st
                        emit_interleaved(pp)
                    else:
                        fill.extend(consume_pieces(ic, 3, att_part.live[3]))
                        pending.extend(range(ic * 4, ic * 4 + 4))
                        for _ in range(6):
                            if fill:
                                fill.pop(0)()
                        refill()
                att_part.live = {}

                def interleaved(q, ic):
                    """Emit pass q's half-passes around attention step ic."""
                    xt = xtiles_cur
                    att_part(ic, 0)
                    half_pass(q, GROUP_A, xt)
                    att_part(ic, 1)
                    att_part(ic, 2)
                    half_pass(q, GROUP_B, xt)
                    att_part(ic, 3)
                    att_part(ic, 4)

                # ---------- schedule ----------
                with tc.tile_pool(name="xh", bufs=1) as xh0:
                    xtiles_cur = [
                        xh0.tile([128, 1024], bf, tag=f"x{k}", name=f"xa{k}")
                        for k in range(KT)]
                    # quarter-0 x slices first so pass 0 streams at the
                    # DMA delivery rate; weights striped ahead of their use
                    nc.sync.dma_start(out=xtiles_cur[0][:, 0:512],
                                      in_=xt_d[0:128, 0:512])
                    nc.sync.dma_start(out=wk_all[:, 0:4, :],
                                      in_=wk_d[:, 0:512])
                    nc.sync.dma_start(out=wv_all[:, 0:4, :],
                                      in_=wv_d[:, 0:512])
                    nc.sync.dma_start(out=wq_all[:, 0, 0:4, :],
                                      in_=wq_d[:, 0:512])
                    nc.sync.dma_start(out=cb[:], in_=cb_d[:])
                    nc.sync.dma_start(out=onesf[:], in_=onesf_d[:])
                    nc.sync.dma_start(out=wk_all[:, 4:, :],
                                      in_=wk_d[:, 512:KT * 128])
                    nc.sync.dma_start(out=wv_all[:, 4:, :],
                                      in_=wv_d[:, 512:KT * 128])
                    nc.sync.dma_start(out=wq_all[:, 0, 4:, :],
                                      in_=wq_d[:, 512:KT * 128])
                    for k in range(1, 4):
                        nc.sync.dma_start(
                            out=xtiles_cur[k][:, 0:512],
                            in_=xt_d[k * 128:(k + 1) * 128, 0:512])
                    for c in range(1, 4):
                        nc.sync.dma_start(
                            out=wq_all[:, c, :, :],
                            in_=wq_d[:, c * KT * 128:(c + 1) * KT * 128])
                    for k in range(4, KT):
                        nc.sync.dma_start(
                            out=xtiles_cur[k][:, 0:512],
                            in_=xt_d[k * 128:(k + 1) * 128, 0:512])
                    nc.sync.dma_start(out=cosk[:], in_=cosk_d[:])
                    nc.sync.dma_start(out=sink[:], in_=sink_d[:])
                    for k in range(KT):
                        nc.sync.dma_start(
                            out=xtiles_cur[k][:, 512:1024],
                            in_=xt_d[k * 128:(k + 1) * 128, 512:1024])
                    nc.sync.dma_start(out=cosq[:], in_=cosq_d[:])
                    nc.sync.dma_start(out=sinq[:], in_=sinq_d[:])
                    nc.sync.dma_start(out=wo_sb[:], in_=wo_d[:])

                    interleaved(0, -1)
                    interleaved(1, -1)

                with tc.tile_pool(name="xh2", bufs=1) as xh1:
                    xtiles_cur = [
                        xh1.tile([128, 1024], bf, tag=f"x{k}", name=f"xb{k}")
                        for k in range(KT)]
                    for k in range(KT):
                        nc.sync.dma_start(
                            out=xtiles_cur[k][:],
                            in_=xt_d[k * 128:(k + 1) * 128, 1024:2048])

                    interleaved(2, 0)
                    interleaved(3, 1)
                    # remaining attention chunks: ic=3 first so its 2MB of
                    # o_proj output drains while ic=2 still computes
                    for ic in (3, 2):
                        for step in range(5):
                            att_part(ic, step)
                    # drain remaining consume/oproj work
                    while fill or pending:
                        if fill:
                            fill.pop(0)()
                        refill()

    nc.compile()
    return nc


def _get_program():
    global _PROGRAM
    if _PROGRAM is None:
        _PROGRAM = _build_program()
    return _PROGRAM


def _host_consts():
    # rot matrix: out[d', s] = sum_d R[d, d'] t[d, s] = rot(t)[d', s]
    R = np.zeros((128, 128), dtype=np.float32)
    for dp in range(64):
        R[dp + 64, dp] = -1.0
    for dp in range(64, 128):
        R[dp - 64, dp] = 1.0
    # triangular boundary mask: tri[p, u] = 1 where p <= u
    p = np.arange(128)[:, None]
    u = np.arange(128)[None, :]
    tri = (p <= u).astype(np.float32)
    ones = np.ones((128, 128), dtype=np.float32)
    cb = np.concatenate([R, tri, ones], axis=1)
    return np.ascontiguousarray(cb.astype(BF16)), ones.astype(np.float32)


def _prearrange_w(Wslice):
    """[H, M] weight slice -> [128, KT*M] k-tile-major SBUF layout."""
    h, m = Wslice.shape
    assert h == H
    w = Wslice.reshape(KT, 128, m).transpose(1, 0, 2).reshape(128, KT * m)
    return np.ascontiguousarray(w.astype(BF16))


def kernel(x, sin, cos, Wq, Wk, Wv, Wo, q_norm_w, k_norm_w):
    from concourse.bass_utils import run_bass_kernel_spmd

    nc = _get_program()

    qw = np.asarray(q_norm_w, dtype=np.float32)
    kw = np.asarray(k_norm_w, dtype=np.float32)
    cosT = np.ascontiguousarray(np.asarray(cos, np.float32).T)  # [128, S]
    sinT = np.ascontiguousarray(np.asarray(sin, np.float32).T)
    # pre-rotation sin fold: the device computes
    #   fin[e] = qs[e]*cos[e] + (qs*sin_pre)[(e+64)%128]
    # so sin_pre[f] = sinT[(f+64)%128] * w[f] * (+1 if f<64 else -1)
    rowsign = np.where(np.arange(HD) < 64, 1.0, -1.0).astype(np.float32)
    sinT_pre = np.roll(sinT, 64, axis=0) * rowsign[:, None]
    cosqf = (cosT * qw[:, None]).astype(BF16)
    sinqf = (sinT_pre * qw[:, None]).astype(BF16)
    coskf = (cosT * kw[:, None]).astype(BF16)
    sinkf = (sinT_pre * kw[:, None]).astype(BF16)
    cb, onesf = _host_consts()

    x = np.asarray(x, np.float32)
    xts = [np.ascontiguousarray(x[b].T).astype(BF16) for b in range(B)]
    Wq = np.asarray(Wq, np.float32)
    Wk = np.asarray(Wk, np.float32)
    Wv = np.asarray(Wv, np.float32)
    Wo = np.asarray(Wo, np.float32)

    def _wq_headmajor(Wslice):
        # [H, 512] -> [128, GQ*KT*128] head-major k-tile-major layout
        w = (Wslice.reshape(KT, 128, GQ, 128)
             .transpose(1, 2, 0, 3).reshape(128, GQ * KT * 128))
        return np.ascontiguousarray(w.astype(BF16))

    in_maps = []
    for core in range(8):
        b, g = divmod(core, 4)
        in_maps.append(
            {
                "xt": xts[b],
                "wq": _wq_headmajor(Wq[:, g * 512:(g + 1) * 512]),
                "wk": _prearrange_w(Wk[:, g * 128:(g + 1) * 128]),
                "wv": _prearrange_w(Wv[:, g * 128:(g + 1) * 128]),
                "wo": np.ascontiguousarray(
                    Wo[g * 512:(g + 1) * 512, :]
                    .reshape(GQ, 128, H).transpose(1, 0, 2)
                    .reshape(128, GQ * H).astype(BF16)),
                "cosq": cosqf,
                "sinq": sinqf,
                "cosk": coskf,
                "sink": sinkf,
                "cb": cb,
                "onesf": onesf,
            }
        )

    trace = os.environ.get("KERNEL_TRACE", "0") == "1"
    if trace:
        _inject_ntff_hook()
    res = run_bass_kernel_spmd(nc, in_maps, list(range(8)), trace=trace)
    if trace and res.exec_time_ns is not None:
        print(f"HW exec time: {res.exec_time_ns} ns", file=sys.stderr)
        kernel.last_exec_time_ns = res.exec_time_ns

    out = np.zeros((B, S, H), dtype=np.float32)
    for core in range(8):
        b = core // 4
        out[b] += np.asarray(res.results[core]["out"], dtype=np.float32)
    return out


kernel.last_exec_time_ns = None


def _inject_ntff_hook():
    """Recreate antenv.axon_hooks (absent in this image) so
    run_bass_kernel_spmd(trace=True) can capture NTFF profiles."""
    import types
    import contextlib
    import ctypes

    if "antenv.axon_hooks" in sys.modules:
        return
    so_path = "/opt/axon/libaxon_pjrt.so"
    try:
        lib = ctypes.CDLL(so_path)
        lib.axon_start_nrt_profile.argtypes = [
            ctypes.POINTER(ctypes.c_int64),
            ctypes.c_size_t,
        ]
        lib.axon_start_nrt_profile.restype = ctypes.c_int64
        lib.axon_stop_nrt_profile.argtypes = [ctypes.c_char_p]
        lib.axon_stop_nrt_profile.restype = ctypes.c_int64
    except (OSError, AttributeError):
        return

    @contextlib.contextmanager
    def _hook(output_dir, device_ids):
        import jax

        jax.devices()
        if device_ids:
            ids = (ctypes.c_int64 * len(device_ids))(*device_ids)
            rc = lib.axon_start_nrt_profile(ids, len(device_ids))
        else:
            rc = lib.axon_start_nrt_profile(None, 0)
        if rc != 0:
            raise RuntimeError(f"axon_start_nrt_profile rc={rc}")
        try:
            yield
        finally:
            n = lib.axon_stop_nrt_profile(str(output_dir).encode())
            print(f"profile: {n} file(s) -> {output_dir}", file=sys.stderr)

    mod = types.ModuleType("antenv.axon_hooks")
    mod.get_axon_ntff_profile_hook = lambda: _hook
    sys.modules["antenv.axon_hooks"] = mod
